# revision 20
# baseline (speedup 1.0000x reference)
"""GRU decoder Bass kernel for Trainium2, data-parallel over batch on 8 cores.

Math refactoring (exactly equivalent to the reference up to fp assoc.):
  context = hidden[0] is constant across steps, and x_{t} = fc_out_{t-1} is
  linear in [h_t, context].  Folding fc into the input projection:
    gi_t = h_t @ M1.T + CONST          (M1 = W_ih @ fc_W[:, :H], t >= 1)
    gh_t = h_t @ W_hh.T + b_hh
  r/z gates add gi+gh, so P_r = M1_r + W_hh_r, P_z = M1_z + W_hh_z fuse into
  one [4096, 1024] weight:  G_t = h_t @ [P_r | P_z | M1_n | W_hh_n].T + C
  fc_out_t = h_{t+1} @ F1.T + CF with F1 = fc_W[:, :H].
  GRU update in "w-form": w = sigmoid(-pre_z) = 1-z, h' = h + w*(n - h).

Performance structure (v9 = v7 + fp8 gate operands):
  - per core B=64 rows, "split layout": [128 parts = 2 hidden-halves x 64
    batch, 512 free].  M=64 matmul pairs are col-tiled (auto tile_position
    (0,0)/(0,64)) and stream concurrently -> G = 16384 effective columns,
    the 1 col/cycle PE floor.  (fp8 DoubleRow was measured NOT faster for
    M=64: DR is restricted to dst partitions 0-63, so it cannot col-pair,
    and a single DR matmul matches the paired-fp16 rate exactly.)
  - gate matmuls use fp8 e4m3 operands at the regular 1 col/cycle rate:
    same cycle count as fp16 but ~half the PE datapath switching power.
    The steady-state clock sag (2.4 -> ~2.0 GHz P-state under sustained
    power draw) is the dominant run-to-run cost; fp8 aims to avoid it.
    W4 is quantized at scale 256, h at 32; the 8192x-scaled gate PSUM is
    un-scaled inside ACT via the free affine (scale=+-1/8192).
  - PSUM deps are tile-granular, so pre_z is accumulated into TWO separate
    half-banks (z_a cols 0-255, z_b cols 256-511); w = sigmoid(-pre_z)
    halves unblock right after each half-bank completes.
  - bank stream order pre_r, h_n, i_n, z_a, z_b; the loop-carried chain is
    [z_a done] -> w_a -> v_a -> h'_a -> transpose -> fp8 cast -> next G.
  - gate-bank constants are engine-written into PSUM in idle windows
    (has_written bits stay set from the previous step, so start=False MMs
    accumulate on top); all five const writes live on DVE/ACT so the PE
    runs only gates + fc + transposes (~19968 cyc/step).
  - fp16 state + fc; fc deferred one step fills the PE pipe during the
    gate tail; 4x 128x128 fp16 PE transposes feed an ACT fp16 copy (for
    fc) and a DVE x32 fp8 cast (for the next gates, critical path).
"""
import os
import numpy as np

H = 1024
OUT = 768
BATCH = 512
NCORES = 8
B = BATCH // NCORES  # 64

SW = 256.0       # W4 fp8 scale
SH = 32.0        # h fp8 scale
SGATE = SW * SH  # gate PSUM scale (8192)

_BUILD_CACHE = {}

# K-chunk m covers contraction dims offs(m) .. offs(m)+127
def _offs(m):
    return 128 * (m // 2) + 512 * (m % 2)


def _build(T: int):
    from contextlib import ExitStack
    from concourse import tile, mybir, bacc

    F8 = mybir.dt.float8e4
    F16 = mybir.dt.float16
    F32 = mybir.dt.float32
    Sig = mybir.ActivationFunctionType.Sigmoid
    Tanh = mybir.ActivationFunctionType.Tanh
    Copy = mybir.ActivationFunctionType.Copy
    INV_S = 1.0 / SGATE

    nc = bacc.Bacc("TRN2", target_bir_lowering=False, debug=False,
                   num_devices=NCORES)

    dram = {}
    def din(name, shape, dt):
        dram[name] = nc.dram_tensor(name, list(shape), dt, kind="ExternalInput").ap()
        return dram[name]

    w8_d = din("W8", [128, 64, 512], F8)
    f1_d = din("F1", [128, 8 * 768], F16)
    ci_d = din("CINIT", [128, 8, 512], F16)
    cst_d = din("CST", [128, 4, 512], F16)
    id2_d = din("IDENT2", [128, 64], F16)
    idt_d = din("IDENTT", [128, 128], F16)
    h0s_d = din("H0S", [128, 512], F16)
    h0t_d = din("H0T16", [128, 512], F16)
    h0t8_d = din("H0T8", [128, 512], F8)
    g0_d = din("G0", [128, 4, 512], F32)
    cf_d = din("CF", [128, 384], F32)
    out_d = nc.dram_tensor("OUT", [T * 128, 384], F32, kind="ExternalOutput").ap()

    with tile.TileContext(nc) as tc:
        with ExitStack() as ctx:
            wpool = ctx.enter_context(tc.tile_pool(name="weights", bufs=1))
            state = ctx.enter_context(tc.tile_pool(name="state", bufs=2))
            tmp = ctx.enter_context(tc.tile_pool(name="tmp", bufs=2))
            gps = ctx.enter_context(tc.tile_pool(name="gpsum", bufs=1, space="PSUM"))
            fps = ctx.enter_context(tc.tile_pool(name="fpsum", bufs=1, space="PSUM"))
            tps = ctx.enter_context(tc.tile_pool(name="tpsum", bufs=2, space="PSUM"))

            w8 = wpool.tile([128, 64, 512], F8, name="w8")
            f1 = wpool.tile([128, 8 * 768], F16, name="f1")
            ci = wpool.tile([128, 8, 512], F16, name="ci")
            cst = wpool.tile([128, 4, 512], F16, name="cst")
            id2 = wpool.tile([128, 64], F16, name="id2")
            idt = wpool.tile([128, 128], F16, name="idt")
            g0 = wpool.tile([128, 4, 512], F32, name="g0")
            cf = wpool.tile([128, 384], F32, name="cf")

            h0 = state.tile([128, 512], F16, name="h0", tag="h")
            hT0 = state.tile([128, 512], F16, name="hT0", tag="hT16")
            hT80 = state.tile([128, 512], F8, name="hT80", tag="hT8")

            # DMA order: step-0's inputs first (g0/h0/identities/consts) so
            # the pipeline starts while the big W8/F1 loads stream in
            for t_sb, t_d in ((g0, g0_d), (h0, h0s_d), (id2, id2_d),
                              (idt, idt_d), (hT0, h0t_d), (hT80, h0t8_d),
                              (ci, ci_d), (cst, cst_d), (cf, cf_d),
                              (w8, w8_d), (f1, f1_d)):
                nc.sync.dma_start(t_sb[:], t_d[:])

            # gate PSUM banks: pre_r, h_n, i_n full; pre_z as two half-bank
            # tiles so each half's consumers unblock independently
            gb0 = gps.tile([128, 512], F32, name="gb0", tag="gb0")
            gb3 = gps.tile([128, 512], F32, name="gb3", tag="gb3")
            gb2 = gps.tile([128, 512], F32, name="gb2", tag="gb2")
            za = gps.tile([128, 256], F32, name="za", tag="za")
            zb = gps.tile([128, 256], F32, name="zb", tag="zb")

            def g_dest(jj, hf=0):
                # returns (tile, col offset within the logical 512-wide bank)
                if jj == 0: return gb0, 0
                if jj == 3: return gb3, 0
                if jj == 2: return gb2, 0
                return (za, 0) if hf == 0 else (zb, 256)

            def emit_init_mm(jjs):
                # PE-matmul constant init (prologue only): hi+lo fp16 rows
                for jj in jjs:
                    for g in range(2):
                        cc = jj * 2 + g
                        if jj == 1:
                            for hf in range(2):
                                dst, off = g_dest(1, hf)
                                nc.tensor.matmul(
                                    dst[64 * g:64 * (g + 1), :], id2[:, :],
                                    ci[:, cc, off:off + 256],
                                    start=True, stop=False)
                        else:
                            dst, _ = g_dest(jj)
                            nc.tensor.matmul(
                                dst[64 * g:64 * (g + 1), :], id2[:, :],
                                ci[:, cc, :],
                                start=True, stop=False)

            def emit_init_eng(jj, eng, hf=None):
                # engine-written constants: has_written bits stay set from
                # the previous step's matmuls, so start=False MMs accumulate
                if jj == 1:
                    dst, off = g_dest(1, hf)
                    s = cst[:, 1, off:off + 256]
                    ap = dst[:, :]
                else:
                    dst, _ = g_dest(jj)
                    s = cst[:, jj, :]
                    ap = dst[:, :]
                if eng == "act":
                    nc.scalar.activation(ap, s, Copy)
                else:
                    nc.vector.tensor_copy(ap, s)

            def emit_init_pe(jj, hf=None):
                # per-step const rewrite via PE id2 pair (start=True resets
                # the bank); lands in the PE's tail idle window
                if jj == 1:
                    dst, off = g_dest(1, hf)
                    for g in range(2):
                        nc.tensor.matmul(
                            dst[64 * g:64 * (g + 1), :], id2[:, :],
                            ci[:, 2 + g, off:off + 256],
                            start=True, stop=False)
                else:
                    for g in range(2):
                        cc = jj * 2 + g
                        dst, _ = g_dest(jj)
                        nc.tensor.matmul(
                            dst[64 * g:64 * (g + 1), :], id2[:, :],
                            ci[:, cc, :], start=True, stop=False)

            def emit_G_bank(hT8, jj, hf, skip_gc):
                dst, off = g_dest(jj, hf)
                nn = 256 if jj == 1 else 512
                for m in range(8):
                    lhsT = hT8[:, m * 64:(m + 1) * 64]
                    for g in range(2):
                        cc = jj * 2 + g
                        nc.tensor.matmul(
                            dst[64 * g:64 * (g + 1), :], lhsT,
                            w8[:, m * 8 + cc, off:off + nn],
                            start=False, stop=(m == 7),
                            skip_group_check=skip_gc)

            def emit_gates(t, h_prev, pr, pz_h, pin, phn, eng_init, sc):
                r = tmp.tile([128, 512], F16, name=f"r{t}", tag="r")
                t1 = tmp.tile([128, 512], F16, name=f"t1{t}", tag="t1")
                t2 = tmp.tile([128, 512], F16, name=f"t2{t}", tag="t2")
                n = tmp.tile([128, 512], F16, name=f"n{t}", tag="n")
                dd = tmp.tile([128, 512], F16, name=f"d{t}", tag="d")
                w = tmp.tile([128, 512], F16, name=f"w{t}", tag="w")
                v = tmp.tile([128, 512], F16, name=f"v{t}", tag="v")
                h_new = state.tile([128, 512], F16, name=f"h{t}", tag="h")

                nc.scalar.activation(r[:], pr, Sig, scale=sc)
                if eng_init:
                    emit_init_eng(0, "dve")           # c_r: DVE mid-step slack
                nc.vector.tensor_mul(t1[:], r[:], phn)
                if eng_init:
                    emit_init_eng(3, "dve")           # c_hn after t1
                for hf in range(2):
                    s = slice(256 * hf, 256 * (hf + 1))
                    nc.vector.tensor_add(t2[:, s], t1[:, s], pin(hf))
                    nc.scalar.activation(n[:, s], t2[:, s], Tanh, scale=sc)
                    nc.vector.tensor_sub(dd[:, s], n[:, s], h_prev[:, s])
                for hf in range(2):
                    s = slice(256 * hf, 256 * (hf + 1))
                    nc.scalar.activation(w[:, s], pz_h(hf), Sig, scale=-sc)
                    nc.vector.tensor_mul(v[:, s], w[:, s], dd[:, s])
                    nc.vector.tensor_add(h_new[:, s], h_prev[:, s], v[:, s])
                return h_new

            def emit_transpose_pair(t, h_new, hT16_new, hT8_new, i):
                # transposes 2i and 2i+1 into one PSUM tile; fp8 x32 cast on
                # DVE (critical path for next gates) + fp16 copy on ACT (fc)
                trp = tps.tile([128, 256], F16, name=f"trp{t}_{i}", tag="trp")
                for k in range(2):
                    j = 2 * i + k
                    nc.tensor.transpose(trp[:, 128 * k:128 * (k + 1)],
                                        h_new[:, 128 * j:128 * (j + 1)],
                                        idt[:, :])
                # fp8 x32 cast on ACT (critical path: gates wait on it; ACT
                # is idle after w_b), fp16 copy on DVE (fc is a step behind)
                nc.scalar.activation(hT8_new[:, 256 * i:256 * (i + 1)],
                                     trp[:, :], Copy, scale=SH)
                nc.vector.tensor_copy(hT16_new[:, 256 * i:256 * (i + 1)],
                                      trp[:, :])

            def emit_fc(t, hT16, ms):
                for m in ms:
                    lhsT = hT16[:, m * 64:(m + 1) * 64]
                    for g in range(2):
                        nc.tensor.matmul(
                            fcs[t][64 * g:64 * (g + 1), :], lhsT,
                            f1[:, m * 768 + g * 384: m * 768 + g * 384 + 384],
                            start=(m == 0), stop=(m == 7))

            def emit_fc_out(t):
                st = tmp.tile([128, 384], F32, name=f"st{t}", tag="st")
                nc.vector.tensor_add(st[:], fcs[t][:], cf[:])
                nc.sync.dma_start(out_d[t * 128:(t + 1) * 128, :], st[:])

            fcs = {}

            h_prev, hT_prev, hT8_prev = h0, hT0, hT80
            for t in range(T):
                eng_init = t > 0 and t + 1 < T
                if t > 0:
                    skip_gc = t > 1
                    emit_G_bank(hT8_prev, 0, 0, skip_gc)
                    emit_G_bank(hT8_prev, 3, 0, skip_gc)
                    emit_G_bank(hT8_prev, 2, 0, skip_gc)
                    emit_G_bank(hT8_prev, 1, 0, skip_gc)
                    emit_G_bank(hT8_prev, 1, 1, skip_gc)
                    h_new = emit_gates(
                        t, h_prev,
                        gb0[:, :],
                        lambda hf: (za if hf == 0 else zb)[:, :],
                        lambda hf: gb2[:, 256 * hf:256 * (hf + 1)],
                        gb3[:, :], eng_init, INV_S)
                else:
                    h_new = emit_gates(
                        t, h_prev,
                        g0[:, 0, :],
                        lambda hf: g0[:, 1, 256 * hf:256 * (hf + 1)],
                        lambda hf: g0[:, 2, 256 * hf:256 * (hf + 1)],
                        g0[:, 3, :], False, 1.0)

                hT_new = state.tile([128, 512], F16, name=f"hT{t}", tag="hT16")
                hT8_new = state.tile([128, 512], F8, name=f"hT8{t}", tag="hT8")

                if t == 0:
                    if T > 1:
                        emit_init_mm([0, 3, 2, 1])

                if t > 0:
                    fcs[t - 1] = fps.tile([128, 384], F32, name=f"fcp{t-1}",
                                          tag="fcp")
                    emit_fc(t - 1, hT_prev, [0, 1, 2, 3, 4, 5])
                if eng_init:
                    emit_init_pe(2)                   # c_in: on PE so it can
                                                      # never block w_a on ACT
                emit_transpose_pair(t, h_new, hT_new, hT8_new, 0)
                if t > 0:
                    emit_fc(t - 1, hT_prev, [6, 7])
                if eng_init:
                    emit_init_pe(1, hf=0)             # c_z_a (after w_a)
                emit_transpose_pair(t, h_new, hT_new, hT8_new, 1)
                if t > 0:
                    emit_fc_out(t - 1)
                if eng_init:
                    # c_z_b on ACT, priority-deflated (runs in post-w_b idle)
                    with tc.high_priority(offset=-(1 << 20)):
                        emit_init_eng(1, "act", hf=1)

                h_prev, hT_prev, hT8_prev = h_new, hT_new, hT8_new

            fcs[T - 1] = fps.tile([128, 384], F32, name=f"fcp{T-1}", tag="fcp")
            emit_fc(T - 1, hT_prev, list(range(8)))
            emit_fc_out(T - 1)

    nc.compile()
    return nc


def _hi_lo(x):
    hi = x.astype(np.float16)
    lo = (x - hi.astype(np.float32)).astype(np.float16)
    return hi, lo


def _q8(x, scale):
    import ml_dtypes
    return np.clip(x * scale, -240.0, 240.0).astype(ml_dtypes.float8_e4m3)


def kernel(src, hidden, W_ih, W_hh, b_ih, b_hh, fc_W, fc_b, output_len):
    from concourse import bass_utils
    import ml_dtypes

    T = int(output_len)
    src = np.asarray(src, np.float32)
    hidden = np.asarray(hidden, np.float32)
    W_ih = np.asarray(W_ih, np.float32)
    W_hh = np.asarray(W_hh, np.float32)
    b_ih = np.asarray(b_ih, np.float32)
    b_hh = np.asarray(b_hh, np.float32)
    fc_W = np.asarray(fc_W, np.float32)
    fc_b = np.asarray(fc_b, np.float32)

    ctx = hidden[0]          # [B, H]
    h0 = hidden[0]
    x0 = src[0]              # [B, OUT]

    # ---- host weight folding (fp32) ----
    M1 = W_ih @ fc_W[:, :H]          # [3H, H]
    M2 = W_ih @ fc_W[:, H:]          # [3H, H]
    P_r = M1[0:H] + W_hh[0:H]
    P_z = M1[H:2 * H] + W_hh[H:2 * H]
    Wbig4 = np.concatenate([P_r, P_z, M1[2 * H:], W_hh[2 * H:]], axis=0)  # [4096, H]
    F1 = fc_W[:, :H]                 # [OUT, H]

    CONST = ctx @ M2.T + (fc_b @ W_ih.T + b_ih)     # [B, 3H]
    c_r = CONST[:, 0:H] + b_hh[0:H]
    c_z = CONST[:, H:2 * H] + b_hh[H:2 * H]
    c_in = CONST[:, 2 * H:]
    c_hn = np.broadcast_to(b_hh[2 * H:], (BATCH, H)).astype(np.float32)
    CALL = np.stack([c_r, c_z, c_in, c_hn], axis=1) * SGATE  # [B, 4, H] scaled

    CF = ctx @ fc_W[:, H:].T + fc_b                  # [B, OUT]

    gi0 = x0 @ W_ih.T + b_ih
    gh0 = h0 @ W_hh.T + b_hh
    G0_parts = np.stack([gi0[:, :H] + gh0[:, :H],
                         gi0[:, H:2 * H] + gh0[:, H:2 * H],
                         gi0[:, 2 * H:],
                         gh0[:, 2 * H:]], axis=1)    # [B, 4, H]

    # ---- shared (replicated) tensors, K-chunk order m: dims offs(m)+p ----
    # W8[p, m*8+cc, c] = q8(W4[cc*512+c, offs(m)+p] * SW)
    W4q = np.clip(Wbig4 * SW, -240.0, 240.0)         # [4096, 1024] fp32 scaled
    W8s = np.empty((128, 64, 512), np.float32)
    for m in range(8):
        o = _offs(m)
        # block [4096, 128] -> [128, 8, 512]
        W8s[:, m * 8:(m + 1) * 8, :] = W4q[:, o:o + 128].T.reshape(128, 8, 512)
    W8s = W8s.astype(ml_dtypes.float8_e4m3)
    F1r = F1.T.reshape(H, 2, 384)                    # [k, g, c]
    F1s = np.empty((128, 8, 2, 384), np.float32)
    for m in range(8):
        o = _offs(m)
        F1s[:, m] = F1r[o:o + 128]
    F1s = F1s.reshape(128, 8 * 768).astype(np.float16)
    ID2 = np.concatenate([np.eye(64), np.eye(64)], axis=0).astype(np.float16)
    IDT = np.eye(128).astype(np.float16)

    key = T
    if key not in _BUILD_CACHE:
        _BUILD_CACHE[key] = _build(T)
    nc = _BUILD_CACHE[key]

    in_maps = []
    for c in range(NCORES):
        sl = slice(c * B, (c + 1) * B)
        # CINIT: [p, cc, c]: p<64 hi, p>=64 lo of CALL[b, jj, 512g+c]
        call_c = CALL[sl].reshape(B, 4, 2, 512)      # [b, jj, g, c]
        hi, lo = _hi_lo(call_c)
        ci = np.concatenate([hi, lo], axis=0)        # [128, 4, 2, 512]
        ci = np.ascontiguousarray(ci).reshape(128, 8, 512)

        # CST: [64g+b, jj, c] = CALL[b, jj, 512g+c]  (fp16)
        cst = np.ascontiguousarray(
            call_c.transpose(2, 0, 1, 3)).reshape(128, 4, 512).astype(np.float16)

        h0_c = h0[sl]
        H0S = np.concatenate([h0_c[:, :512], h0_c[:, 512:]], axis=0)
        # H0T16[p, 64m + b] = h0[b, offs(m)+p]
        H0T = np.ascontiguousarray(
            h0_c.T.reshape(2, 4, 128, B).transpose(2, 1, 0, 3)).reshape(128, 512)
        H0T8 = _q8(H0T, SH)

        # G0: [64g+b, jj, c] = G0_parts[b, jj, 512g+c]  (fp32, unscaled)
        g0_c = G0_parts[sl].reshape(B, 4, 2, 512)    # [b, jj, g, c]
        G0s = np.ascontiguousarray(
            g0_c.transpose(2, 0, 1, 3)).reshape(128, 4, 512)

        cf_c = CF[sl].reshape(B, 2, 384)             # [b, g, c]
        CFs = np.ascontiguousarray(cf_c.transpose(1, 0, 2)).reshape(128, 384)

        in_maps.append({
            "W8": W8s, "F1": F1s,
            "CINIT": np.ascontiguousarray(ci).astype(np.float16),
            "CST": cst,
            "IDENT2": ID2, "IDENTT": IDT,
            "H0S": np.ascontiguousarray(H0S).astype(np.float16),
            "H0T16": H0T.astype(np.float16),
            "H0T8": H0T8,
            "G0": G0s.astype(np.float32),
            "CF": CFs.astype(np.float32),
        })

    trace = bool(os.environ.get("GRU_TRACE"))
    res = bass_utils.run_bass_kernel_spmd(
        nc, in_maps, core_ids=list(range(NCORES)), trace=trace)
    if trace:
        kernel.last_exec_time_ns = res.exec_time_ns
        kernel.last_results = res

    outs = []
    for c in range(NCORES):
        o = np.asarray(res.results[c]["OUT"], np.float32)  # [T*128, 384]
        o = o.reshape(T, 2, B, 384).transpose(0, 2, 1, 3).reshape(T, B, OUT)
        outs.append(o)
    return np.concatenate(outs, axis=1)              # [T, BATCH, OUT]
